# revision 2
# baseline (speedup 1.0000x reference)
"""LocallyConnected2d (3x3, stride 1) Trainium2 Bass kernel.

Shapes: x [64,32,64,64] f32, weight [1,64,32,62,62,9] f32 -> out [64,64,62,62] f32.

Strategy (v2 — "x-stationary"):
  - Shard output rows (OH=62, padded to 64) across 8 cores: 8 rows/core.
  - For output row h and input column c, the patch tile
    S[(ki,i), b] = x[b, i, h+ki, c] is the matmul STATIONARY operand
    (one LDWEIGHTS per (h,c): 512/core instead of 1488 in the per-location
    formulation), and the weights are the MOVING operand:
      rhs[(ki,i), (j,o)] = W[o, i, h, w=c-2+j, ki, kj=2-j]
    A single matmul thus contributes to up to 3 adjacent output columns
    (w = c-2..c) at once: psum[b, (w,o)] += S.T @ rhs, N up to 192.
  - Each output column w accumulates its 3 kj contributions from matmuls at
    c = w, w+1, w+2 via per-element PSUM has_written accumulate semantics;
    start=True only on the chronologically-first matmul touching each bank.
  - PSUM tile = one bank = [b=64 parts (upper half), 8 w x 64 o = 512 f32];
    matmuls whose 3-column window straddles a bank boundary are split in two.
  - fp16 operands (rel err ~3e-4), fp16 output DMA (cast back on host).
"""

import sys

if "/opt/trn_rl_repo" not in sys.path:
    sys.path.insert(0, "/opt/trn_rl_repo")

import numpy as np

B = 64
CIN = 32
H = W = 64
OH = OW = 62
COUT = 64
NCORES = 8
RH = 8  # padded output rows per core (8*8=64 >= 62)

MODE = "fp16"
TRACE = False
LAST = None

_PROGRAMS = {}


def _build_program(repeat=1, mode=None):
    mode = mode or MODE
    import concourse.bacc as bacc
    import concourse.mybir as mybir
    from concourse.tile import TileContext

    fp32 = mybir.dt.float32
    hdt = mybir.dt.float16 if mode == "fp16" else mybir.dt.bfloat16
    nc = bacc.Bacc(
        "TRN2", target_bir_lowering=False, debug=False, num_devices=NCORES
    )

    wt = nc.declare_dram_parameter("wt", [RH, 96, 64, 3, COUT], hdt, isOutput=False)
    xs = nc.declare_dram_parameter("xs", [RH, 96, 64, B], hdt, isOutput=False)
    out = nc.declare_dram_parameter("out", [RH, B, OW * COUT], hdt, isOutput=True)

    # last input column c that touches psum bank t (w = 8t..8t+7, w <= 61)
    last_c = [min(8 * t + 7, OW - 1) + 2 for t in range(8)]

    with TileContext(nc) as tc:
        with (
            tc.tile_pool(name="wp", bufs=2) as wp,
            tc.tile_pool(name="xp", bufs=2) as xp,
            tc.tile_pool(name="op", bufs=2) as op,
            tc.tile_pool(name="pp", bufs=4, space="PSUM") as pp,
        ):
            for h in [hh for _ in range(repeat) for hh in range(RH)]:
                wtile = wp.tile([96, 64, 3, COUT], hdt, tag="w")
                nc.sync.dma_start(out=wtile[:], in_=wt[h])
                xtile = xp.tile([96, 64, B], hdt, tag="x")
                nc.sync.dma_start(out=xtile[:], in_=xs[h])
                otile = op.tile([128, OW * COUT], hdt, tag="o")
                pstiles = {}
                for c in range(64):
                    j_lo = max(0, 2 - c)
                    j_hi = min(2, 63 - c)
                    w_lo = c - 2 + j_lo
                    w_hi = c - 2 + j_hi
                    t_lo, t_hi = w_lo // 8, w_hi // 8
                    if t_lo == t_hi:
                        segs = [(t_lo, w_lo, w_hi)]
                    else:
                        wb = 8 * t_hi
                        segs = [(t_lo, w_lo, wb - 1), (t_hi, wb, w_hi)]
                    for (t, wl, wh) in segs:
                        jl = wl - (c - 2)
                        jh = wh - (c - 2)
                        n0 = (wl - 8 * t) * COUT
                        n1 = (wh + 1 - 8 * t) * COUT
                        if t not in pstiles:
                            pstiles[t] = pp.tile([128, 512], fp32, tag="ps")
                            start = True
                        else:
                            start = False
                        stop = c == last_c[t]
                        nc.tensor.matmul(
                            pstiles[t][64:128, n0:n1],
                            lhsT=xtile[:, c, :],
                            rhs=wtile[:, c, jl : jh + 1, :],
                            start=start,
                            stop=stop,
                            skip_group_check=True,
                        )
                        if stop:
                            nw = min(8, OW - 8 * t)
                            nc.vector.tensor_copy(
                                otile[64:128, 8 * t * COUT : (8 * t + nw) * COUT],
                                pstiles[t][64:128, 0 : nw * COUT],
                            )
                nc.sync.dma_start(out=out[h], in_=otile[64:128])
    nc.compile()
    return nc


def _prep_inputs(x, weight, mode=None):
    mode = mode or MODE
    import ml_dtypes

    hdt = np.float16 if mode == "fp16" else ml_dtypes.bfloat16

    x = np.ascontiguousarray(x, dtype=np.float32)
    weight = np.ascontiguousarray(weight, dtype=np.float32)

    xpad = np.zeros((B, CIN, H + 2, W), np.float32)
    xpad[:, :, :H, :] = x

    # wt[h, (ki,i), c, j, o] = W[o, i, h, w=c-2+j, ki, kj=2-j]
    w6 = weight[0].reshape(COUT, CIN, OH, OW, 3, 3)  # o,i,h,w,ki,kj
    wtfull = np.zeros((NCORES * RH, 3, CIN, 64, 3, COUT), hdt)
    for j in range(3):
        kj = 2 - j
        sub = np.transpose(w6[:, :, :, :, :, kj], (2, 4, 1, 3, 0))  # h,ki,i,w,o
        wtfull[:OH, :, :, 2 - j : 2 - j + OW, j, :] = sub
    wtfull = wtfull.reshape(NCORES * RH, 96, 64, 3, COUT)

    in_maps = []
    for core in range(NCORES):
        r0 = RH * core
        xw = xpad[:, :, r0 : r0 + RH + 2, :]  # [b,i,RH+2,c]
        sv = np.lib.stride_tricks.sliding_window_view(xw, 3, axis=2)  # b,i,h,c,ki
        xs_c = np.transpose(sv, (2, 4, 1, 3, 0))  # h,ki,i,c,b
        xs_c = np.ascontiguousarray(xs_c.reshape(RH, 96, 64, B).astype(hdt))
        in_maps.append(
            {
                "wt": np.ascontiguousarray(wtfull[r0 : r0 + RH]),
                "xs": xs_c,
            }
        )
    return in_maps


def kernel(x, weight):
    global LAST
    from concourse.bass_utils import run_bass_kernel_spmd

    if MODE not in _PROGRAMS:
        _PROGRAMS[MODE] = _build_program(mode=MODE)
    in_maps = _prep_inputs(np.asarray(x), np.asarray(weight))
    res = run_bass_kernel_spmd(
        _PROGRAMS[MODE], in_maps, list(range(NCORES)), trace=TRACE
    )
    LAST = res
    full = np.concatenate([r["out"] for r in res.results], axis=0)  # [64h,b,3968]
    full = full.reshape(NCORES * RH, B, OW, COUT).astype(np.float32)
    return np.ascontiguousarray(np.transpose(full[:OH], (1, 3, 0, 2)))


# revision 3
# speedup vs baseline: 923.0289x; 923.0289x over previous
"""LocallyConnected2d (3x3, stride 1) Trainium2 Bass kernel.

Shapes: x [64,32,64,64] f32, weight [1,64,32,62,62,9] f32 -> out [64,64,62,62] f32.

Strategy (v2 — "x-stationary"):
  - Shard output rows (OH=62, padded to 64) across 8 cores: 8 rows/core.
  - For output row h and input column c, the patch tile
    S[(ki,i), b] = x[b, i, h+ki, c] is the matmul STATIONARY operand
    (one LDWEIGHTS per (h,c): 512/core instead of 1488 in the per-location
    formulation), and the weights are the MOVING operand:
      rhs[(ki,i), (j,o)] = W[o, i, h, w=c-2+j, ki, kj=2-j]
    A single matmul thus contributes to up to 3 adjacent output columns
    (w = c-2..c) at once: psum[b, (w,o)] += S.T @ rhs, N up to 192.
  - Each output column w accumulates its 3 kj contributions from matmuls at
    c = w, w+1, w+2 via per-element PSUM has_written accumulate semantics;
    start=True only on the chronologically-first matmul touching each bank.
  - PSUM tile = one bank = [b=64 parts (upper half), 8 w x 64 o = 512 f32];
    matmuls whose 3-column window straddles a bank boundary are split in two.
  - fp16 operands (rel err ~3e-4), fp16 output DMA (cast back on host).
"""

import sys

if "/opt/trn_rl_repo" not in sys.path:
    sys.path.insert(0, "/opt/trn_rl_repo")

import numpy as np

B = 64
CIN = 32
H = W = 64
OH = OW = 62
COUT = 64
NCORES = 8
RH = 8  # padded output rows per core (8*8=64 >= 62)

MODE = "fp16"
TRACE = False
LAST = None

_PROGRAMS = {}


def _build_program(repeat=1, mode=None):
    mode = mode or MODE
    import concourse.bacc as bacc
    import concourse.mybir as mybir
    from concourse.tile import TileContext

    fp32 = mybir.dt.float32
    hdt = mybir.dt.float16 if mode == "fp16" else mybir.dt.bfloat16
    nc = bacc.Bacc(
        "TRN2", target_bir_lowering=False, debug=False, num_devices=NCORES
    )

    wt = nc.declare_dram_parameter("wt", [RH, 96, 64, 3, COUT], hdt, isOutput=False)
    xs = nc.declare_dram_parameter("xs", [RH, 96, 64, B], hdt, isOutput=False)
    out = nc.declare_dram_parameter("out", [RH, B, OW * COUT], hdt, isOutput=True)

    # last input column c that touches psum bank t (w = 8t..8t+7, w <= 61)
    last_c = [min(8 * t + 7, OW - 1) + 2 for t in range(8)]

    with TileContext(nc) as tc:
        with (
            tc.tile_pool(name="wp", bufs=2) as wp,
            tc.tile_pool(name="xp", bufs=2) as xp,
            tc.tile_pool(name="op", bufs=2) as op,
            tc.tile_pool(name="pp", bufs=4, space="PSUM") as pp,
        ):
            for h in [hh for _ in range(repeat) for hh in range(RH)]:
                wtile = wp.tile([96, 64, 3, COUT], hdt, tag="w")
                nc.sync.dma_start(out=wtile[:], in_=wt[h])
                xtile = xp.tile([96, 64, B], hdt, tag="x")
                nc.sync.dma_start(out=xtile[:], in_=xs[h])
                otile = op.tile([128, OW * COUT], hdt, tag="o")
                pstiles = {}
                for c in range(64):
                    j_lo = max(0, 2 - c)
                    j_hi = min(2, 63 - c)
                    w_lo = c - 2 + j_lo
                    w_hi = c - 2 + j_hi
                    t_lo, t_hi = w_lo // 8, w_hi // 8
                    if t_lo == t_hi:
                        segs = [(t_lo, w_lo, w_hi)]
                    else:
                        wb = 8 * t_hi
                        segs = [(t_lo, w_lo, wb - 1), (t_hi, wb, w_hi)]
                    for (t, wl, wh) in segs:
                        jl = wl - (c - 2)
                        jh = wh - (c - 2)
                        n0 = (wl - 8 * t) * COUT
                        n1 = (wh + 1 - 8 * t) * COUT
                        if t not in pstiles:
                            pstiles[t] = pp.tile(
                                [128, 512], fp32, tag="ps", name=f"ps_{h}_{t}"
                            )
                            start = True
                        else:
                            start = False
                        stop = c == last_c[t]
                        nc.tensor.matmul(
                            pstiles[t][64:128, n0:n1],
                            lhsT=xtile[:, c, :],
                            rhs=wtile[:, c, jl : jh + 1, :],
                            start=start,
                            stop=stop,
                            skip_group_check=True,
                        )
                        if stop:
                            nw = min(8, OW - 8 * t)
                            nc.vector.tensor_copy(
                                otile[64:128, 8 * t * COUT : (8 * t + nw) * COUT],
                                pstiles[t][64:128, 0 : nw * COUT],
                            )
                nc.sync.dma_start(out=out[h], in_=otile[64:128])
    nc.compile()
    return nc


def _prep_inputs(x, weight, mode=None):
    mode = mode or MODE
    import ml_dtypes

    hdt = np.float16 if mode == "fp16" else ml_dtypes.bfloat16

    x = np.ascontiguousarray(x, dtype=np.float32)
    weight = np.ascontiguousarray(weight, dtype=np.float32)

    xpad = np.zeros((B, CIN, H + 2, W), np.float32)
    xpad[:, :, :H, :] = x

    # wt[h, (ki,i), c, j, o] = W[o, i, h, w=c-2+j, ki, kj=2-j]
    w6 = weight[0].reshape(COUT, CIN, OH, OW, 3, 3)  # o,i,h,w,ki,kj
    wtfull = np.zeros((NCORES * RH, 3, CIN, 64, 3, COUT), hdt)
    for j in range(3):
        kj = 2 - j
        sub = np.transpose(w6[:, :, :, :, :, kj], (2, 4, 1, 3, 0))  # h,ki,i,w,o
        wtfull[:OH, :, :, 2 - j : 2 - j + OW, j, :] = sub
    wtfull = wtfull.reshape(NCORES * RH, 96, 64, 3, COUT)

    in_maps = []
    for core in range(NCORES):
        r0 = RH * core
        xw = xpad[:, :, r0 : r0 + RH + 2, :]  # [b,i,RH+2,c]
        sv = np.lib.stride_tricks.sliding_window_view(xw, 3, axis=2)  # b,i,h,c,ki
        xs_c = np.transpose(sv, (2, 4, 1, 3, 0))  # h,ki,i,c,b
        xs_c = np.ascontiguousarray(xs_c.reshape(RH, 96, 64, B).astype(hdt))
        in_maps.append(
            {
                "wt": np.ascontiguousarray(wtfull[r0 : r0 + RH]),
                "xs": xs_c,
            }
        )
    return in_maps


def kernel(x, weight):
    global LAST
    from concourse.bass_utils import run_bass_kernel_spmd

    if MODE not in _PROGRAMS:
        _PROGRAMS[MODE] = _build_program(mode=MODE)
    in_maps = _prep_inputs(np.asarray(x), np.asarray(weight))
    res = run_bass_kernel_spmd(
        _PROGRAMS[MODE], in_maps, list(range(NCORES)), trace=TRACE
    )
    LAST = res
    full = np.concatenate([r["out"] for r in res.results], axis=0)  # [64h,b,3968]
    full = full.reshape(NCORES * RH, B, OW, COUT).astype(np.float32)
    return np.ascontiguousarray(np.transpose(full[:OH], (1, 3, 0, 2)))


# revision 13
# speedup vs baseline: 1308.0636x; 1.4171x over previous
"""LocallyConnected2d (3x3, stride 1) Trainium2 Bass kernel.

Shapes: x [64,32,64,64] f32, weight [1,64,32,62,62,9] f32 -> out [64,64,62,62] f32.

Strategy (v2 — "x-stationary"):
  - Shard output rows (OH=62, padded to 64) across 8 cores: 8 rows/core.
  - For output row h and input column c, the patch tile
    S[(ki,i), b] = x[b, i, h+ki, c] is the matmul STATIONARY operand
    (one LDWEIGHTS per (h,c): 512/core instead of 1488 in the per-location
    formulation), and the weights are the MOVING operand:
      rhs[(ki,i), (j,o)] = W[o, i, h, w=c-2+j, ki, kj=2-j]
    A single matmul thus contributes to up to 3 adjacent output columns
    (w = c-2..c) at once: psum[b, (w,o)] += S.T @ rhs, N up to 192.
  - Each output column w accumulates its 3 kj contributions from matmuls at
    c = w, w+1, w+2 via per-element PSUM has_written accumulate semantics;
    start=True only on the chronologically-first matmul touching each bank.
  - PSUM tile = one bank = [b=64 parts (upper half), 8 w x 64 o = 512 f32];
    matmuls whose 3-column window straddles a bank boundary are split in two.
  - fp16 operands (rel err ~3e-4), fp16 output DMA (cast back on host).
"""

import sys

if "/opt/trn_rl_repo" not in sys.path:
    sys.path.insert(0, "/opt/trn_rl_repo")

import numpy as np

B = 64
CIN = 32
H = W = 64
OH = OW = 62
COUT = 64
NCORES = 8
RH = 8  # padded output rows per core (8*8=64 >= 62)

MODE = "fp16"
TRACE = False
LAST = None

_PROGRAMS = {}


def _build_program(repeat=1, mode=None, loop=None, probe=None):
    """repeat: unrolled in-program repetitions of the 8-row body.
    loop: if set, additionally wrap in a hardware For_i loop (for timing).
    probe: None | 'nomm' (skip matmuls+copies) | 'wt1' (load wt[0] only)."""
    mode = mode or MODE
    import concourse.bacc as bacc
    import concourse.mybir as mybir
    from concourse.tile import TileContext
    from contextlib import nullcontext

    fp32 = mybir.dt.float32
    hdt = mybir.dt.float16 if mode == "fp16" else mybir.dt.bfloat16
    nc = bacc.Bacc(
        "TRN2", target_bir_lowering=False, debug=False, num_devices=NCORES
    )

    wt = nc.declare_dram_parameter("wt", [RH, 96, 64, 3, COUT], hdt, isOutput=False)
    xs = nc.declare_dram_parameter("xs", [RH, 96, 64, B], hdt, isOutput=False)
    out = nc.declare_dram_parameter("out", [RH, B, OW * COUT], hdt, isOutput=True)

    # last input column c that touches psum bank t (w = 8t..8t+7, w <= 61)
    last_c = [min(8 * t + 7, OW - 1) + 2 for t in range(8)]

    with TileContext(nc) as tc:
        loop_cm = (
            tc.For_i(0, loop, 1, hint_engines=(mybir.EngineType.PE,))
            if loop is not None
            else nullcontext()
        )
        with (
            tc.tile_pool(name="wp", bufs=2) as wp,
            tc.tile_pool(name="xp", bufs=2) as xp,
            tc.tile_pool(name="op", bufs=2) as op,
            tc.tile_pool(name="pp", bufs=4, space="PSUM") as pp,
            loop_cm,
        ):
            wtile_shared = None
            if probe == "wt1":
                wtile_shared = wp.tile([96, 64, 3, COUT], hdt, tag="w")
                nc.sync.dma_start(out=wtile_shared[:], in_=wt[0])
            for h in [hh for _ in range(repeat) for hh in range(RH)]:
                if probe == "wt1":
                    wtile = wtile_shared
                else:
                    wtile = wp.tile([96, 64, 3, COUT], hdt, tag="w")
                    nc.sync.dma_start(out=wtile[:], in_=wt[h])
                xtile = xp.tile([96, 64, B], hdt, tag="x")
                nc.sync.dma_start(out=xtile[:], in_=xs[h])
                otile = op.tile([128, OW * COUT], hdt, tag="o")
                pstiles = {}
                for c in range(64 if probe != "nomm" else 0):
                    j_lo = max(0, 2 - c)
                    j_hi = min(2, 63 - c)
                    w_lo = c - 2 + j_lo
                    w_hi = c - 2 + j_hi
                    t_lo, t_hi = w_lo // 8, w_hi // 8
                    if t_lo == t_hi:
                        segs = [(t_lo, w_lo, w_hi)]
                    else:
                        wb = 8 * t_hi
                        segs = [(t_lo, w_lo, wb - 1), (t_hi, wb, w_hi)]
                    for (t, wl, wh) in segs:
                        jl = wl - (c - 2)
                        jh = wh - (c - 2)
                        n0 = (wl - 8 * t) * COUT
                        n1 = (wh + 1 - 8 * t) * COUT
                        if t not in pstiles:
                            pstiles[t] = pp.tile(
                                [128, 512], fp32, tag="ps", name=f"ps_{h}_{t}"
                            )
                            start = True
                        else:
                            start = False
                        stop = c == last_c[t]
                        nc.tensor.matmul(
                            pstiles[t][64:128, n0:n1],
                            lhsT=xtile[:, c, :],
                            rhs=wtile[:, c, jl : jh + 1, :],
                            start=start,
                            stop=stop,
                            skip_group_check=True,
                        )
                        if stop:
                            nw = min(8, OW - 8 * t)
                            nc.vector.tensor_copy(
                                otile[64:128, 8 * t * COUT : (8 * t + nw) * COUT],
                                pstiles[t][64:128, 0 : nw * COUT],
                            )
                if probe == "nomm":
                    # pure DMA probe: ship xtile bytes out instead of results
                    nc.sync.dma_start(out=out[h], in_=xtile[0:64, 0:62, :])
                else:
                    nc.sync.dma_start(out=out[h], in_=otile[64:128])
    nc.compile()
    return nc


def _build_program_pair(repeat=1, mode=None, loop=None):
    """v3: process output rows in even/odd pairs. Even row's matmuls write
    PSUM partitions 0-63 (PE col groups 0-1), odd row's write 64-127 (col
    groups 2-3) — disjoint column strips stream concurrently in the array."""
    mode = mode or MODE
    import concourse.bacc as bacc
    import concourse.mybir as mybir
    from concourse.tile import TileContext
    from contextlib import nullcontext

    fp32 = mybir.dt.float32
    hdt = mybir.dt.float16 if mode == "fp16" else mybir.dt.bfloat16
    nc = bacc.Bacc(
        "TRN2", target_bir_lowering=False, debug=False, num_devices=NCORES
    )

    wt = nc.declare_dram_parameter("wt", [RH, 96, 64, 3, COUT], hdt, isOutput=False)
    xs = nc.declare_dram_parameter("xs", [RH, 96, 64, B], hdt, isOutput=False)
    out = nc.declare_dram_parameter(
        "out", [RH // 2, 128, OW * COUT], hdt, isOutput=True
    )

    last_c = [min(8 * t + 7, OW - 1) + 2 for t in range(8)]

    def segs_for(c):
        j_lo = max(0, 2 - c)
        j_hi = min(2, 63 - c)
        w_lo = c - 2 + j_lo
        w_hi = c - 2 + j_hi
        t_lo, t_hi = w_lo // 8, w_hi // 8
        if t_lo == t_hi:
            return [(t_lo, w_lo, w_hi)]
        wb = 8 * t_hi
        return [(t_lo, w_lo, wb - 1), (t_hi, wb, w_hi)]

    with TileContext(nc) as tc:
        loop_cm = (
            tc.For_i(0, loop, 1, hint_engines=(mybir.EngineType.PE,))
            if loop is not None
            else nullcontext()
        )
        with (
            tc.tile_pool(name="wp", bufs=3) as wp,
            tc.tile_pool(name="xp", bufs=3) as xp,
            tc.tile_pool(name="op", bufs=2) as op,
            tc.tile_pool(name="pp", bufs=6, space="PSUM") as pp,
        ):
            with loop_cm:
                for hp in [
                    hh for _ in range(repeat) for hh in range(RH // 2)
                ]:
                    tiles = []
                    for par in range(2):
                        h = 2 * hp + par
                        wtile = wp.tile([96, 64, 3, COUT], hdt, tag="w")
                        nc.sync.dma_start(out=wtile[:], in_=wt[h])
                        xtile = xp.tile([96, 64, B], hdt, tag="x")
                        nc.sync.dma_start(out=xtile[:], in_=xs[h])
                        tiles.append((wtile, xtile, {}))
                    otile = op.tile([128, OW * COUT], hdt, tag="o")
                    for c in range(64):
                        for par in range(2):
                            h = 2 * hp + par
                            wtile, xtile, pstiles = tiles[par]
                            p0 = 64 * par
                            for (t, wl, wh) in segs_for(c):
                                jl = wl - (c - 2)
                                jh = wh - (c - 2)
                                n0 = (wl - 8 * t) * COUT
                                n1 = (wh + 1 - 8 * t) * COUT
                                if t not in pstiles:
                                    pstiles[t] = pp.tile(
                                        [128, 512],
                                        fp32,
                                        tag="ps",
                                        name=f"ps_{hp}_{par}_{t}",
                                    )
                                    start = True
                                else:
                                    start = False
                                stop = c == last_c[t]
                                nc.tensor.matmul(
                                    pstiles[t][p0 : p0 + 64, n0:n1],
                                    lhsT=xtile[:, c, :],
                                    rhs=wtile[:, c, jl : jh + 1, :],
                                    start=start,
                                    stop=stop,
                                    skip_group_check=True,
                                )
                                if stop:
                                    nw = min(8, OW - 8 * t)
                                    nc.vector.tensor_copy(
                                        otile[
                                            p0 : p0 + 64,
                                            8 * t * COUT : (8 * t + nw) * COUT,
                                        ],
                                        pstiles[t][p0 : p0 + 64, 0 : nw * COUT],
                                    )
                    nc.sync.dma_start(out=out[hp], in_=otile[:])
    nc.compile()
    return nc


def _prep_inputs(x, weight, mode=None):
    mode = mode or MODE
    import ml_dtypes

    hdt = np.float16 if mode == "fp16" else ml_dtypes.bfloat16

    x = np.ascontiguousarray(x, dtype=np.float32)
    weight = np.ascontiguousarray(weight, dtype=np.float32)

    xpad = np.zeros((B, CIN, H + 2, W), np.float32)
    xpad[:, :, :H, :] = x

    # wt[h, (ki,i), c, j, o] = W[o, i, h, w=c-2+j, ki, kj=2-j]
    w6 = weight[0].reshape(COUT, CIN, OH, OW, 3, 3)  # o,i,h,w,ki,kj
    wtfull = np.zeros((NCORES * RH, 3, CIN, 64, 3, COUT), hdt)
    for j in range(3):
        kj = 2 - j
        sub = np.transpose(w6[:, :, :, :, :, kj], (2, 4, 1, 3, 0))  # h,ki,i,w,o
        wtfull[:OH, :, :, 2 - j : 2 - j + OW, j, :] = sub
    wtfull = wtfull.reshape(NCORES * RH, 96, 64, 3, COUT)

    in_maps = []
    for core in range(NCORES):
        r0 = RH * core
        xw = xpad[:, :, r0 : r0 + RH + 2, :]  # [b,i,RH+2,c]
        sv = np.lib.stride_tricks.sliding_window_view(xw, 3, axis=2)  # b,i,h,c,ki
        xs_c = np.transpose(sv, (2, 4, 1, 3, 0))  # h,ki,i,c,b
        xs_c = np.ascontiguousarray(xs_c.reshape(RH, 96, 64, B).astype(hdt))
        in_maps.append(
            {
                "wt": np.ascontiguousarray(wtfull[r0 : r0 + RH]),
                "xs": xs_c,
            }
        )
    return in_maps


PAIR = True


def kernel(x, weight):
    global LAST
    from concourse.bass_utils import run_bass_kernel_spmd

    key = (MODE, PAIR)
    if key not in _PROGRAMS:
        build = _build_program_pair if PAIR else _build_program
        _PROGRAMS[key] = build(mode=MODE)
    in_maps = _prep_inputs(np.asarray(x), np.asarray(weight))
    res = run_bass_kernel_spmd(
        _PROGRAMS[key], in_maps, list(range(NCORES)), trace=TRACE
    )
    LAST = res
    full = np.concatenate([r["out"] for r in res.results], axis=0)
    full = full.reshape(NCORES * RH, B, OW, COUT).astype(np.float32)
    return np.ascontiguousarray(np.transpose(full[:OH], (1, 3, 0, 2)))
